# revision 35
# baseline (speedup 1.0000x reference)
"""Trainium2 Bass kernel for nn_DAInsHead (moe_routing).

Per-row hard-routed 3-layer MLP: rows with levels[i]==l get
    out[i] = W3[l].T @ relu(W2[l].T @ relu(W1[l].T @ x[i] + b1[l]) + b2[l]) + b3[l]

Strategy (vs the reference's dense 4x-redundant masked compute):
  * Host: stable-sort rows by level, deal each level's rows evenly to the 8
    cores, pad each (core, level) segment to cap = ceil(count/8) (exact, no
    128-rounding -- rows are the matmul FREE dim, so no alignment needed),
    and transpose to feature-major xT [D, R_core]. All matmul operands are
    cast to bf16 on the host (total rel err ~4e-3, well under the 2e-2 gate).
  * W3 is folded into W2 on the host: relu(z)*w3 = max(z*w3, 0) for w3>0
    and min(z*w3, 0) for w3<0, so with W2' = W2*w3 (columns sorted by
    sign(w3), positives first) the L2 eviction directly produces each
    column's final contribution; b2' = b2*w3 rides along. This removes the
    entire L3 matvec (8x 512-cycle M=1 matmuls per tile, ~55us/core of PE
    time) from the TensorE.
  * Device (identical SPMD program on 8 cores): per level, keep W1/W2'
    resident in SBUF and stream ~456-row tiles: L1/L2 are K=8-chunk
    accumulated 128x128xN bf16 matmuls (1 cycle/row). L1 evicts
    relu(acc+b1) split DVE/ACT. L2 evicts e_mc = clamp(acc + b2') with
    Relu-bias on ACT for the positive-sign partition range and
    tensor_scalar(ADD,MIN) on DVE for the negative range (one mixed chunk
    gets both). DVE then folds the 8 e chunks into s[128, rt] (7 adds), and
    a single ones[128,1] matmul (N cycles, vs 8N for the old L3) reduces s
    over partitions into out[1, rt]; partial results DMA out per tile.
  * Host: scatter per-core outputs back to original row order, add b3.

Measured on 8xTRN2 (this problem's shapes): 1038us (f32r baseline) -> 968us
(routed bf16, L3 on PE) -> this version targets ~890us: PE does only the
L1+L2 roofline stream (row_count x 128 cyc) plus one N-cycle reduce per
tile. Known non-wins (measured earlier): fp8 DoubleRow needs 3-term error
compensation to pass 2e-2 which costs more matmuls than it saves; mid-tile
L3 interleave stalls the in-order PE behind fresh evictions.
"""
import os
import sys

sys.path.insert(0, "/opt/trn_rl_repo")

import ml_dtypes
import numpy as np

import concourse.bacc as bacc
import concourse.mybir as mybir
import concourse.tile as tile
from concourse.bass_utils import run_bass_kernel_spmd

F32 = mybir.dt.float32
BF16 = mybir.dt.bfloat16
ADD = mybir.AluOpType.add
MAX = mybir.AluOpType.max
MIN = mybir.AluOpType.min
RELU = mybir.ActivationFunctionType.Relu

NC = 8          # cores
L = 4           # levels
D = 1024        # in features
H = 1024        # hidden
KC = D // 128   # contraction chunks
MC = H // 128   # output-feature chunks

LAST_RESULTS = None       # BassKernelResults of the most recent run (for test.py)
_PROGRAM_CACHE = {}


def _row_tiles(cap):
    """Split a per-level capacity into near-equal row tiles of <=512 (PSUM
    bank limit for f32 accumulation)."""
    if cap <= 0:
        return []
    nt = -(-cap // 512)
    base, rem = divmod(cap, nt)
    return [base + 1] * rem + [base] * (nt - rem)


def _level_tiles(lvl, cap):
    """Tile list for a level: level 0 leads with a small tile (fewer bytes
    gate the first matmul during the DMA clock ramp); the last level ends
    with a small tile (the end-of-kernel reduce waits on a short s chain)."""
    if lvl == 0 and cap > 512:
        return [256] + _row_tiles(cap - 256)
    if lvl == L - 1 and cap > 512:
        return _row_tiles(cap - 256) + [256]
    return _row_tiles(cap)


def _build_program(caps, n_pos, zero_b2):
    """Build + compile the SPMD program for per-level capacities `caps` and
    per-level positive-w3 column counts `n_pos` (W2 columns are host-sorted
    so cols [0, n_pos) have w3>=0 and [n_pos, H) have w3<0). `zero_b2`
    selects a 1-op per-partition clamp for the one sign-mixed chunk per
    level (engine APs must start 32-aligned, so the chunk can't be split at
    an arbitrary partition)."""
    r_core = sum(caps)
    nc = bacc.Bacc("TRN2", target_bir_lowering=False, debug=False, num_devices=NC)
    xT = nc.dram_tensor("xT", [D, r_core], BF16, kind="ExternalInput")
    W1 = nc.dram_tensor("W1", [L, D, H], BF16, kind="ExternalInput")
    W2 = nc.dram_tensor("W2", [L, H, H], BF16, kind="ExternalInput")  # pre-folded W2*w3, sign-sorted
    b1 = nc.dram_tensor("b1", [L, H], F32, kind="ExternalInput")
    b2 = nc.dram_tensor("b2", [L, H], F32, kind="ExternalInput")      # pre-folded b2*w3, sign-sorted
    clo = nc.dram_tensor("clo", [L, H], F32, kind="ExternalInput")    # 0 where w3>=0 else -BIG
    chi = nc.dram_tensor("chi", [L, H], F32, kind="ExternalInput")    # +BIG where w3>=0 else 0
    msk = nc.dram_tensor("msk", [L, H], F32, kind="ExternalInput")    # 1 where w3>=0 else 0
    # Per-level partition-reduce results: row j = tile j's [1, rt] sums,
    # padded to 512 (junk beyond each tile's width; host slices valid parts).
    # Per-tile DMA of the full s[128, rt] for a host-side reduce was tried
    # and measured +45ns on EVERY matmul (SBUF-read DMA steals moving-operand
    # bandwidth) -- keep output DMA tiny.
    NT_MAX = max((len(_level_tiles(l, c)) for l, c in enumerate(caps) if c),
                 default=1)
    out = nc.dram_tensor("out", [L, NT_MAX, 512], F32, kind="ExternalOutput")

    xT_r = xT.rearrange("(kc p) r -> p kc r", p=128)  # [128, KC, r_core]

    with tile.TileContext(nc) as tc:
        with (
            tc.tile_pool(name="wpool", bufs=2) as wpool,
            tc.tile_pool(name="bpool", bufs=2) as bpool,
            tc.tile_pool(name="xpool", bufs=2) as xpool,
            tc.tile_pool(name="hpool", bufs=2) as hpool,
            tc.tile_pool(name="epool", bufs=10) as epool,
            tc.tile_pool(name="spool", bufs=2) as spool,
            tc.tile_pool(name="opool", bufs=4) as opool,
            tc.tile_pool(name="ps", bufs=8, space="PSUM") as ps,
        ):
            ones = bpool.tile([128, NT_MAX], BF16, tag="ones", bufs=1)
            nc.gpsimd.memset(ones[:], 1.0)

            # Deferred per-LEVEL partition reduce: level l's ones-matmuls are
            # emitted between level l+1's first-tile L1 and L2 groups, so the
            # in-order PE never waits on the DVE s chains. The nt matmuls for
            # a level run back-to-back into DISJOINT partition rows of one
            # PSUM bank (start only on the first: start=True clears the whole
            # bank), so the ~0.3us enter/exit issue hiccup around M=1
            # matmuls is paid once per level, not once per tile.
            pending = {}

            def flush_pending():
                if not pending:
                    return
                s_lv, p_lvl, tlist, parity = pending.pop("v")
                # One bank per tile-reduce, output at partition 0, plain
                # start&stop semantics. (Packing 3 strips at 0/32/64 into a
                # shared bank with start only on the first was tried:
                # start=True clears has_written only for the written region,
                # so the start=False strips accumulated onto stale PSUM.)
                toff = 0
                for j, rt_j in enumerate(tlist):
                    b_ps = ps.tile([128, 512], F32, tag="acc", name="acc")
                    nc.tensor.matmul(b_ps[0:1, 0:rt_j], ones[:, j:j + 1],
                                     s_lv[:, toff:toff + rt_j],
                                     start=True, stop=True)
                    o_t = opool.tile([1, 512], F32, tag="o", name="o")
                    if (j + parity) % 2 == 0:
                        nc.vector.tensor_scalar(
                            o_t[0:1, :], b_ps[0:1, 0:512], 0.0, None, ADD)
                    else:
                        nc.scalar.copy(o_t[0:1, :], b_ps[0:1, 0:512])
                    q = nc.gpsimd if (j + parity) % 2 == 0 else nc.sync
                    q.dma_start(out[p_lvl][j:j + 1, :], o_t[0:1, :])
                    toff += rt_j

            off = 0
            n_tile = 0
            for lvl in range(L):
                cap = caps[lvl]
                if cap == 0:
                    continue
                tiles_l = _level_tiles(lvl, cap)
                # For level 0, issue the first row-tile's x DMA before the
                # weight DMAs so the PE can start as soon as the first weight
                # chunk lands instead of waiting behind 8.5MB of weights.
                pre_x = None
                if lvl == 0:
                    rt0 = tiles_l[0]
                    pre_x = xpool.tile([128, KC, rt0], BF16, tag="x")
                    # first x chunk only -- the very first matmul needs just
                    # this chunk plus w1k[0]'s first column chunk, so those
                    # DMAs go first
                    nc.sync.dma_start(pre_x[:, 0, :], xT_r[:, 0, 0:rt0])
                w1k = []
                w2k = []
                t1 = wpool.tile([128, H], BF16, tag="w1k0")
                if lvl == 0:
                    # per-column-chunk DMAs: the very first matmul needs only
                    # the first 32KB of W1 (subtile deps unblock it early)
                    for mcq in range(MC):
                        nc.sync.dma_start(t1[:, mcq * 128:(mcq + 1) * 128],
                                          W1[lvl][0:128, mcq * 128:(mcq + 1) * 128])
                else:
                    nc.sync.dma_start(t1[:], W1[lvl][0:128, :])
                w1k.append(t1)
                if lvl == 0:
                    for kc in range(1, KC):
                        nc.sync.dma_start(pre_x[:, kc, :], xT_r[:, kc, 0:rt0])
                # Tiny bias tiles before the bulk of W1/W2 so evictions
                # never wait behind 8MB of weight DMA.
                b1t = bpool.tile([128, MC], F32, tag="b1")
                nc.sync.dma_start(b1t[:], b1[lvl].rearrange("(mc p) -> p mc", p=128))
                b2t = bpool.tile([128, MC], F32, tag="b2")
                nc.sync.dma_start(b2t[:], b2[lvl].rearrange("(mc p) -> p mc", p=128))
                mix_mc = n_pos[lvl] // 128 if n_pos[lvl] % 128 else -1
                if mix_mc >= 0:
                    if zero_b2:
                        clot = bpool.tile([128, 1], F32, tag="clo")
                        nc.sync.dma_start(
                            clot[:], clo[lvl].rearrange("(mc p) -> p mc", p=128)[:, mix_mc:mix_mc + 1])
                        chit = bpool.tile([128, 1], F32, tag="chi")
                        nc.sync.dma_start(
                            chit[:], chi[lvl].rearrange("(mc p) -> p mc", p=128)[:, mix_mc:mix_mc + 1])
                    else:
                        mskt = bpool.tile([128, 1], F32, tag="msk")
                        nc.sync.dma_start(
                            mskt[:], msk[lvl].rearrange("(mc p) -> p mc", p=128)[:, mix_mc:mix_mc + 1])
                # Per-kc weight tiles so the first matmuls only wait on the
                # first 256KB of weight DMA, and level l+1 prefetch
                # double-buffers against level l (bufs=2 per tag).
                for kc in range(1, KC):
                    t1 = wpool.tile([128, H], BF16, tag=f"w1k{kc}")
                    nc.sync.dma_start(t1[:], W1[lvl][kc * 128:(kc + 1) * 128, :])
                    w1k.append(t1)
                for kc in range(KC):
                    t2 = wpool.tile([128, H], BF16, tag=f"w2k{kc}")
                    nc.sync.dma_start(t2[:], W2[lvl][kc * 128:(kc + 1) * 128, :])
                    w2k.append(t2)

                # one level-wide s accumulator; each tile's chain fills its
                # column slice, the level flush reduces it over partitions
                s_lv = spool.tile([128, cap], BF16, tag="s", name="s")
                toff_l = 0
                for ti, rt in enumerate(tiles_l):
                    if pre_x is not None and ti == 0:
                        x_t = pre_x
                    else:
                        x_t = xpool.tile([128, KC, rt], BF16, tag="x")
                        # per-kc chunk DMAs: the tile's first matmul needs
                        # only chunk 0, and the chunks interleave with the
                        # next level's 4MB weight prefetch on the shared
                        # queue instead of queueing behind it
                        for kc in range(KC):
                            nc.sync.dma_start(x_t[:, kc, :],
                                              xT_r[:, kc, off:off + rt])

                    # L1 runs kc-outer in two 4-bank halves: the first matmul
                    # only depends on w1k[0] + x_t, so the PE ramps with the
                    # weight DMA stream instead of waiting for all of W1.
                    # The very first tile uses all 8 banks in one kc-outer
                    # pass (8 matmuls per weight chunk instead of 4) to halve
                    # the weight-arrival rate the ramping DMA must sustain.
                    h1 = hpool.tile([128, MC, rt], BF16, tag="h1")
                    groups = [range(MC)] if (lvl == 0 and ti == 0) else [
                        range(0, 4), range(4, 8)]
                    for mcs in groups:
                        accs = {mc: ps.tile([128, rt], F32, tag="acc", name="acc")
                                for mc in mcs}
                        for kc in range(KC):
                            for mc in mcs:
                                nc.tensor.matmul(
                                    accs[mc][:], w1k[kc][:, mc * 128:(mc + 1) * 128],
                                    x_t[:, kc, :], start=(kc == 0), stop=(kc == KC - 1))
                        # evictions split DVE/ACT so PSUM banks release fast
                        # (bank reuse gates the next matmul group's start);
                        # DVE takes 1 of 4 since it also carries the L2
                        # e-chunk accumulation below
                        for mc in mcs:
                            if mc % 4 == 0:
                                nc.vector.tensor_scalar(
                                    h1[:, mc, :], accs[mc][:], b1t[:, mc:mc + 1], 0.0, ADD, MAX)
                            else:
                                nc.scalar.activation(
                                    h1[:, mc, :], accs[mc][:], RELU, bias=b1t[:, mc:mc + 1])

                    # previous level's partition reduce goes here, between
                    # this level's first L1 and L2 matmul groups on the PE
                    if ti == 0:
                        flush_pending()

                    # L2 with W3 folded in: eviction yields each column's
                    # final signed contribution; DVE folds chunks into s.
                    # Engine APs must start at 32-aligned partitions, so
                    # chunks are never partition-split: sign-pure chunks use
                    # Relu-with-bias (ACT) / (ADD,MIN) (DVE); the one mixed
                    # chunk uses a per-partition (MAX lo, MIN hi) clamp
                    # (b2==0) or a full-width masked blend (general b2).
                    # Per half: all PSUM evictions are emitted BEFORE the s
                    # adds, so bank release (which gates the next matmul
                    # group) isn't queued behind adds on the in-order DVE.
                    s_sl = s_lv[:, toff_l:toff_l + rt]
                    chain = {"init": False, "e": None}

                    def acc_s(e_t, chain=chain, s_sl=s_sl):
                        if not chain["init"] and chain["e"] is None:
                            chain["e"] = e_t
                        elif not chain["init"]:
                            nc.vector.tensor_tensor(s_sl, chain["e"][:], e_t[:], ADD)
                            chain["init"] = True
                            chain["e"] = None
                        else:
                            nc.vector.tensor_tensor(s_sl, s_sl, e_t[:], ADD)

                    for half in range(2):
                        mcs = range(4 * half, 4 * half + 4)
                        accs = {mc: ps.tile([128, rt], F32, tag="acc", name="acc")
                                for mc in mcs}
                        for kc in range(MC):
                            for mc in mcs:
                                nc.tensor.matmul(
                                    accs[mc][:], w2k[kc][:, mc * 128:(mc + 1) * 128],
                                    h1[:, kc, :], start=(kc == 0), stop=(kc == MC - 1))
                        to_add = []
                        for mc in mcs:
                            tpos = min(max(n_pos[lvl] - mc * 128, 0), 128)
                            e_t = epool.tile([128, rt], BF16, tag="e", name="e")
                            if tpos == 128:    # pure w3>=0: relu(z*w3) on ACT
                                nc.scalar.activation(
                                    e_t[:], accs[mc][:], RELU, bias=b2t[:, mc:mc + 1])
                                to_add.append(e_t)
                            elif tpos == 0:    # pure w3<0: min(z*w3, 0) on DVE
                                nc.vector.tensor_scalar(
                                    e_t[:], accs[mc][:], b2t[:, mc:mc + 1], 0.0, ADD, MIN)
                                to_add.append(e_t)
                            elif zero_b2:      # mixed: per-partition clamp
                                nc.vector.tensor_scalar(
                                    e_t[:], accs[mc][:], clot[:, 0:1], chit[:, 0:1],
                                    MAX, MIN)
                                to_add.append(e_t)
                            else:              # mixed, general b2: blend
                                e_p = epool.tile([128, rt], BF16, tag="e", name="e")
                                nc.scalar.activation(
                                    e_p[:], accs[mc][:], RELU, bias=b2t[:, mc:mc + 1])
                                nc.vector.tensor_scalar(
                                    e_t[:], accs[mc][:], b2t[:, mc:mc + 1], 0.0, ADD, MIN)
                                d_t = epool.tile([128, rt], BF16, tag="e", name="e")
                                nc.vector.tensor_tensor(d_t[:], e_p[:], e_t[:],
                                                        mybir.AluOpType.subtract)
                                nc.vector.tensor_scalar(d_t[:], d_t[:], mskt[:, 0:1],
                                                        None, mybir.AluOpType.mult)
                                to_add.append(e_t)
                                to_add.append(d_t)
                        for e_t in to_add:
                            acc_s(e_t)

                    off += rt
                    toff_l += rt
                    n_tile += 1
                pending["v"] = (s_lv, lvl, tiles_l, lvl % 2)
            flush_pending()
    nc.compile()
    return nc


def kernel(x, levels, W1, b1, W2, b2, W3, b3):
    global LAST_RESULTS
    x = np.ascontiguousarray(np.asarray(x, dtype=np.float32))
    levels = np.asarray(levels)
    n = x.shape[0]

    # --- host-side routing: sort rows by level, deal evenly to cores ---
    order = np.argsort(levels, kind="stable")
    counts = np.bincount(np.asarray(levels, dtype=np.int64), minlength=L)[:L]

    # per-level capacity shared by all cores: exactly ceil(count/8); rows are
    # the matmul free dim so no 128-alignment is needed
    caps = [int(-(-int(counts[lvl]) // NC)) for lvl in range(L)]
    r_core = sum(caps)

    # fold W3 into W2/b2, sign-sorted (positives first) per level
    BIG = np.float32(3.0e38)
    W3f = np.asarray(W3, dtype=np.float32)      # [L, H, 1]
    W2f = np.asarray(W2, dtype=np.float32)      # [L, H, H]
    b2f = np.asarray(b2, dtype=np.float32)      # [L, H]
    W2p = np.empty_like(W2f)
    b2p = np.empty_like(b2f)
    clo = np.empty_like(b2f)
    chi = np.empty_like(b2f)
    msk = np.empty_like(b2f)
    n_pos = []
    for lvl in range(L):
        w3 = W3f[lvl, :, 0]
        pos = w3 >= 0
        perm = np.concatenate([np.where(pos)[0], np.where(~pos)[0]])
        n_pos.append(int(pos.sum()))
        W2p[lvl] = (W2f[lvl] * w3[None, :])[:, perm]
        b2p[lvl] = (b2f[lvl] * w3)[perm]
        posp = pos[perm]
        clo[lvl] = np.where(posp, np.float32(0), -BIG)
        chi[lvl] = np.where(posp, BIG, np.float32(0))
        msk[lvl] = posp.astype(np.float32)
    zero_b2 = not np.any(b2p)

    # per-core padded index lists + validity masks
    idx = np.zeros((NC, r_core), dtype=np.int64)
    valid = np.zeros((NC, r_core), dtype=bool)
    lvl_start = np.concatenate([[0], np.cumsum(counts)])
    seg_off = 0
    for lvl in range(L):
        rows = order[lvl_start[lvl]:lvl_start[lvl + 1]]
        nl = len(rows)
        q, rem = divmod(nl, NC)
        start = 0
        for c in range(NC):
            cnt = q + (1 if c < rem else 0)
            idx[c, seg_off:seg_off + cnt] = rows[start:start + cnt]
            valid[c, seg_off:seg_off + cnt] = True
            start += cnt
        seg_off += caps[lvl]

    key = (tuple(caps), tuple(n_pos), zero_b2)
    nc = _PROGRAM_CACHE.get(key)
    if nc is None:
        nc = _build_program(caps, n_pos, zero_b2)
        _PROGRAM_CACHE[key] = nc

    W1b = np.asarray(W1, dtype=np.float32).astype(ml_dtypes.bfloat16)
    W2b = W2p.astype(ml_dtypes.bfloat16)
    b1f = np.asarray(b1, dtype=np.float32)
    in_maps = []
    for c in range(NC):
        xTc = np.ascontiguousarray(x[idx[c]].T).astype(ml_dtypes.bfloat16)  # [D, r_core]
        in_maps.append({
            "xT": xTc,
            "W1": W1b,
            "W2": W2b,
            "b1": b1f,
            "b2": b2p,
            "clo": clo,
            "chi": chi,
            "msk": msk,
        })

    trace = bool(os.environ.get("BASS_KERNEL_TRACE"))
    try:
        res = run_bass_kernel_spmd(nc, in_maps, core_ids=list(range(NC)), trace=trace)
    except Exception:
        # transient NRT_EXEC_UNIT_UNRECOVERABLE wedges have been observed to
        # clear on the next attempt
        import time
        time.sleep(5)
        res = run_bass_kernel_spmd(nc, in_maps, core_ids=list(range(NC)), trace=trace)
    LAST_RESULTS = res

    result = np.zeros((n, 1), dtype=np.float32)
    for c in range(NC):
        oarr = np.asarray(res.results[c]["out"], dtype=np.float32)  # [L, NT_MAX, 512]
        o = np.zeros(r_core, dtype=np.float32)
        pos = 0
        for lvl in range(L):
            toff = 0
            for j, rt in enumerate(_level_tiles(lvl, caps[lvl])):
                o[pos + toff:pos + toff + rt] = oarr[lvl, j, 0:rt]
                toff += rt
            pos += caps[lvl]
        result[idx[c][valid[c]], 0] = o[valid[c]]
    result += np.asarray(b3, dtype=np.float32)[np.asarray(levels, dtype=np.int64), :]
    return result


# revision 40
# speedup vs baseline: 1.0019x; 1.0019x over previous
"""Trainium2 Bass kernel for nn_DAInsHead (moe_routing).

Per-row hard-routed 3-layer MLP: rows with levels[i]==l get
    out[i] = W3[l].T @ relu(W2[l].T @ relu(W1[l].T @ x[i] + b1[l]) + b2[l]) + b3[l]

Strategy (vs the reference's dense 4x-redundant masked compute):
  * Host: stable-sort rows by level, deal each level's rows evenly to the 8
    cores, pad each (core, level) segment to cap = ceil(count/8) (exact, no
    128-rounding -- rows are the matmul FREE dim, so no alignment needed),
    and transpose to feature-major xT [D, R_core]. All matmul operands are
    cast to bf16 on the host (total rel err ~4e-3, well under the 2e-2 gate).
  * W3 is folded into W2 on the host: relu(z)*w3 = max(z*w3, 0) for w3>0
    and min(z*w3, 0) for w3<0, so with W2' = W2*w3 (columns sorted by
    sign(w3), positives first) the L2 eviction directly produces each
    column's final contribution; b2' = b2*w3 rides along. This removes the
    entire L3 matvec (8x 512-cycle M=1 matmuls per tile, ~55us/core of PE
    time) from the TensorE.
  * Device (identical SPMD program on 8 cores): per level, keep W1/W2'
    resident in SBUF and stream ~457-511-row tiles: L1/L2 are K=8-chunk
    accumulated 128x128xN bf16 matmuls (1 cycle/row + ~6 cyc/instruction).
    L1 evicts relu(acc+b1) split DVE/ACT. L2 evicts the final signed
    contribution per chunk: Relu-bias on ACT for sign-pure-positive chunks,
    tensor_scalar(ADD,MIN) on DVE for pure-negative, and a per-partition
    (MAX lo, MIN hi) clamp for the one sign-mixed chunk (engine APs must
    start 32-aligned, so the chunk can't be partition-split; clamp needs
    b2==0 -- a masked-blend fallback handles general b2). DVE folds the 8 e
    chunks into a level-wide s[128, cap] accumulator (7 adds per tile,
    emitted after each half's evictions so PSUM release isn't queued behind
    adds on the in-order DVE).
  * Partition reduce of s: per LEVEL, nt back-to-back M=1 ones-matmuls
    (N cycles each at 4x column tiling), emitted between the NEXT level's
    first L1 and L2 matmul groups so the in-order PE never waits on the DVE
    s chains (emitting per tile at tile end measured a 3.6us PE stall per
    tile; per-tile emission pipelined one tile later still cost ~0.6us/tile
    in M=1 enter/exit issue hiccups). Each reduce gets its own PSUM bank at
    partition 0 (start=True clears has_written only for the written region,
    so packing strips 0/32/64 in one bank corrupts the start=False strips).
  * Host: scatter per-core outputs back to original row order, add b3.

Measured on 8xTRN2 (this problem's shapes): 1038us (f32r baseline) -> 968us
(routed bf16, L3 on PE) -> 924.5us (this version; PE busy 96.7%, stream at
the N/2.4GHz+2.5ns bf16 issue-pace roofline, ~874us of pure L1+L2 stream).
Known non-wins (measured): fp8 DoubleRow needs 3-term error compensation to
pass 2e-2 which costs more matmuls than it saves; host-side partition
reduce via per-tile DMA of s[128, rt] slows EVERY matmul ~45ns (SBUF-read
DMA steals moving-operand bandwidth); 256-row first/last tiles + kc-outer
8-bank L1 for tile 0 to soften the cold-DMA startup measured +8us net
(per-instruction overhead of small-N matmuls outweighs the startup gain).
"""
import os
import sys

sys.path.insert(0, "/opt/trn_rl_repo")

import ml_dtypes
import numpy as np

import concourse.bacc as bacc
import concourse.mybir as mybir
import concourse.tile as tile
from concourse.bass_utils import run_bass_kernel_spmd

F32 = mybir.dt.float32
BF16 = mybir.dt.bfloat16
ADD = mybir.AluOpType.add
MAX = mybir.AluOpType.max
MIN = mybir.AluOpType.min
RELU = mybir.ActivationFunctionType.Relu

NC = 8          # cores
L = 4           # levels
D = 1024        # in features
H = 1024        # hidden
KC = D // 128   # contraction chunks
MC = H // 128   # output-feature chunks

LAST_RESULTS = None       # BassKernelResults of the most recent run (for test.py)
_PROGRAM_CACHE = {}


def _row_tiles(cap):
    """Split a per-level capacity into near-equal row tiles of <=512 (PSUM
    bank limit for f32 accumulation)."""
    if cap <= 0:
        return []
    nt = -(-cap // 512)
    base, rem = divmod(cap, nt)
    return [base + 1] * rem + [base] * (nt - rem)


def _build_program(caps, n_pos, zero_b2):
    """Build + compile the SPMD program for per-level capacities `caps` and
    per-level positive-w3 column counts `n_pos` (W2 columns are host-sorted
    so cols [0, n_pos) have w3>=0 and [n_pos, H) have w3<0). `zero_b2`
    selects a 1-op per-partition clamp for the one sign-mixed chunk per
    level (engine APs must start 32-aligned, so the chunk can't be split at
    an arbitrary partition)."""
    r_core = sum(caps)
    nc = bacc.Bacc("TRN2", target_bir_lowering=False, debug=False, num_devices=NC)
    xT = nc.dram_tensor("xT", [D, r_core], BF16, kind="ExternalInput")
    W1 = nc.dram_tensor("W1", [L, D, H], BF16, kind="ExternalInput")
    W2 = nc.dram_tensor("W2", [L, H, H], BF16, kind="ExternalInput")  # pre-folded W2*w3, sign-sorted
    b1 = nc.dram_tensor("b1", [L, H], F32, kind="ExternalInput")
    b2 = nc.dram_tensor("b2", [L, H], F32, kind="ExternalInput")      # pre-folded b2*w3, sign-sorted
    clo = nc.dram_tensor("clo", [L, H], F32, kind="ExternalInput")    # 0 where w3>=0 else -BIG
    chi = nc.dram_tensor("chi", [L, H], F32, kind="ExternalInput")    # +BIG where w3>=0 else 0
    msk = nc.dram_tensor("msk", [L, H], F32, kind="ExternalInput")    # 1 where w3>=0 else 0
    # Per-level partition-reduce results: row j = tile j's [1, rt] sums,
    # padded to 512 (junk beyond each tile's width; host slices valid parts).
    # Per-tile DMA of the full s[128, rt] for a host-side reduce was tried
    # and measured +45ns on EVERY matmul (SBUF-read DMA steals moving-operand
    # bandwidth) -- keep output DMA tiny.
    NT_MAX = max(len(_row_tiles(c)) for c in caps if c) if any(caps) else 1
    out = nc.dram_tensor("out", [L, NT_MAX, 512], F32, kind="ExternalOutput")

    xT_r = xT.rearrange("(kc p) r -> p kc r", p=128)  # [128, KC, r_core]

    with tile.TileContext(nc) as tc:
        with (
            tc.tile_pool(name="wpool", bufs=2) as wpool,
            tc.tile_pool(name="bpool", bufs=2) as bpool,
            tc.tile_pool(name="xpool", bufs=2) as xpool,
            tc.tile_pool(name="hpool", bufs=2) as hpool,
            tc.tile_pool(name="epool", bufs=10) as epool,
            tc.tile_pool(name="spool", bufs=2) as spool,
            tc.tile_pool(name="opool", bufs=4) as opool,
            tc.tile_pool(name="ps", bufs=8, space="PSUM") as ps,
        ):
            ones = bpool.tile([128, NT_MAX], BF16, tag="ones", bufs=1)
            nc.gpsimd.memset(ones[:], 1.0)

            # Deferred per-LEVEL partition reduce: level l's ones-matmuls are
            # emitted between level l+1's first-tile L1 and L2 groups, so the
            # in-order PE never waits on the DVE s chains. The nt matmuls for
            # a level run back-to-back into DISJOINT partition rows of one
            # PSUM bank (start only on the first: start=True clears the whole
            # bank), so the ~0.3us enter/exit issue hiccup around M=1
            # matmuls is paid once per level, not once per tile.
            pending = {}

            def flush_pending():
                if not pending:
                    return
                s_lv, p_lvl, tlist, parity = pending.pop("v")
                # One bank per tile-reduce, output at partition 0, plain
                # start&stop semantics. (Packing 3 strips at 0/32/64 into a
                # shared bank with start only on the first was tried:
                # start=True clears has_written only for the written region,
                # so the start=False strips accumulated onto stale PSUM.)
                toff = 0
                for j, rt_j in enumerate(tlist):
                    b_ps = ps.tile([128, 512], F32, tag="acc", name="acc")
                    nc.tensor.matmul(b_ps[0:1, 0:rt_j], ones[:, j:j + 1],
                                     s_lv[:, toff:toff + rt_j],
                                     start=True, stop=True)
                    o_t = opool.tile([1, 512], F32, tag="o", name="o")
                    if (j + parity) % 2 == 0:
                        nc.vector.tensor_scalar(
                            o_t[0:1, :], b_ps[0:1, 0:512], 0.0, None, ADD)
                    else:
                        nc.scalar.copy(o_t[0:1, :], b_ps[0:1, 0:512])
                    q = nc.gpsimd if (j + parity) % 2 == 0 else nc.sync
                    q.dma_start(out[p_lvl][j:j + 1, :], o_t[0:1, :])
                    toff += rt_j

            off = 0
            n_tile = 0
            for lvl in range(L):
                cap = caps[lvl]
                if cap == 0:
                    continue
                tiles_l = _row_tiles(cap)
                # For level 0, issue the first row-tile's x DMA before the
                # weight DMAs so the PE can start as soon as the first weight
                # chunk lands instead of waiting behind 8.5MB of weights.
                pre_x = None
                if lvl == 0:
                    rt0 = tiles_l[0]
                    pre_x = xpool.tile([128, KC, rt0], BF16, tag="x")
                    # first x chunk only -- the very first matmul needs just
                    # this chunk plus w1k[0], so those two DMAs go first
                    nc.sync.dma_start(pre_x[:, 0, :], xT_r[:, 0, 0:rt0])
                # At level 0 (kernel start) each dma_start costs ~0.6us of
                # ISSUE time on its engine queue, so ~26 serial descriptors
                # on the sync queue would gate W2 by ~16us. Spread the
                # startup issues across the idle scalar/vector/gpsimd
                # queues; steady-state levels keep everything on sync
                # (prefetch has a whole level of cover there).
                # (DMA issue is only supported on sync/SP, scalar/ACT and
                # gpsimd queues)
                q_x = nc.gpsimd if lvl == 0 else nc.sync
                q_b = nc.scalar if lvl == 0 else nc.sync
                w1k = []
                w2k = []
                t1 = wpool.tile([128, H], BF16, tag="w1k0")
                nc.sync.dma_start(t1[:], W1[lvl][0:128, :])
                w1k.append(t1)
                if lvl == 0:
                    for kc in range(1, KC):
                        q_x.dma_start(pre_x[:, kc, :], xT_r[:, kc, 0:rt0])
                # Tiny bias tiles before the bulk of W1/W2 so evictions
                # never wait behind 8MB of weight DMA.
                b1t = bpool.tile([128, MC], F32, tag="b1")
                q_b.dma_start(b1t[:], b1[lvl].rearrange("(mc p) -> p mc", p=128))
                b2t = bpool.tile([128, MC], F32, tag="b2")
                q_b.dma_start(b2t[:], b2[lvl].rearrange("(mc p) -> p mc", p=128))
                mix_mc = n_pos[lvl] // 128 if n_pos[lvl] % 128 else -1
                if mix_mc >= 0:
                    if zero_b2:
                        clot = bpool.tile([128, 1], F32, tag="clo")
                        q_b.dma_start(
                            clot[:], clo[lvl].rearrange("(mc p) -> p mc", p=128)[:, mix_mc:mix_mc + 1])
                        chit = bpool.tile([128, 1], F32, tag="chi")
                        q_b.dma_start(
                            chit[:], chi[lvl].rearrange("(mc p) -> p mc", p=128)[:, mix_mc:mix_mc + 1])
                    else:
                        mskt = bpool.tile([128, 1], F32, tag="msk")
                        q_b.dma_start(
                            mskt[:], msk[lvl].rearrange("(mc p) -> p mc", p=128)[:, mix_mc:mix_mc + 1])
                # Per-kc weight tiles so the first matmuls only wait on the
                # first 256KB of weight DMA, and level l+1 prefetch
                # double-buffers against level l (bufs=2 per tag).
                for kc in range(1, KC):
                    t1 = wpool.tile([128, H], BF16, tag=f"w1k{kc}")
                    nc.sync.dma_start(t1[:], W1[lvl][kc * 128:(kc + 1) * 128, :])
                    w1k.append(t1)
                for kc in range(KC):
                    t2 = wpool.tile([128, H], BF16, tag=f"w2k{kc}")
                    if lvl == 0:
                        q_w2 = nc.scalar if kc % 2 == 0 else nc.gpsimd
                    else:
                        q_w2 = nc.sync
                    q_w2.dma_start(t2[:], W2[lvl][kc * 128:(kc + 1) * 128, :])
                    w2k.append(t2)

                # one level-wide s accumulator; each tile's chain fills its
                # column slice, the level flush reduces it over partitions
                s_lv = spool.tile([128, cap], BF16, tag="s", name="s")
                toff_l = 0
                for ti, rt in enumerate(tiles_l):
                    if pre_x is not None and ti == 0:
                        x_t = pre_x
                    else:
                        x_t = xpool.tile([128, KC, rt], BF16, tag="x")
                        # per-kc chunk DMAs: the tile's first matmul needs
                        # only chunk 0, and the chunks interleave with the
                        # next level's 4MB weight prefetch on the shared
                        # queue instead of queueing behind it
                        for kc in range(KC):
                            nc.sync.dma_start(x_t[:, kc, :],
                                              xT_r[:, kc, off:off + rt])

                    # L1 runs kc-outer in two 4-bank halves: the first matmul
                    # only depends on w1k[0] + x_t, so the PE ramps with the
                    # weight DMA stream instead of waiting for all of W1.
                    h1 = hpool.tile([128, MC, rt], BF16, tag="h1")
                    for half in range(2):
                        mcs = range(4 * half, 4 * half + 4)
                        accs = {mc: ps.tile([128, rt], F32, tag="acc", name="acc")
                                for mc in mcs}
                        for kc in range(KC):
                            for mc in mcs:
                                nc.tensor.matmul(
                                    accs[mc][:], w1k[kc][:, mc * 128:(mc + 1) * 128],
                                    x_t[:, kc, :], start=(kc == 0), stop=(kc == KC - 1))
                        # evictions split DVE/ACT so PSUM banks release fast
                        # (bank reuse gates the next matmul group's start);
                        # DVE takes 1 of 4 since it also carries the L2
                        # e-chunk accumulation below
                        for mc in mcs:
                            if mc % 4 == 0:
                                nc.vector.tensor_scalar(
                                    h1[:, mc, :], accs[mc][:], b1t[:, mc:mc + 1], 0.0, ADD, MAX)
                            else:
                                nc.scalar.activation(
                                    h1[:, mc, :], accs[mc][:], RELU, bias=b1t[:, mc:mc + 1])

                    # previous level's partition reduce goes here, between
                    # this level's first L1 and L2 matmul groups on the PE
                    if ti == 0:
                        flush_pending()

                    # L2 with W3 folded in: eviction yields each column's
                    # final signed contribution; DVE folds chunks into s.
                    # Engine APs must start at 32-aligned partitions, so
                    # chunks are never partition-split: sign-pure chunks use
                    # Relu-with-bias (ACT) / (ADD,MIN) (DVE); the one mixed
                    # chunk uses a per-partition (MAX lo, MIN hi) clamp
                    # (b2==0) or a full-width masked blend (general b2).
                    # Per half: all PSUM evictions are emitted BEFORE the s
                    # adds, so bank release (which gates the next matmul
                    # group) isn't queued behind adds on the in-order DVE.
                    s_sl = s_lv[:, toff_l:toff_l + rt]
                    chain = {"init": False, "e": None}

                    def acc_s(e_t, chain=chain, s_sl=s_sl):
                        if not chain["init"] and chain["e"] is None:
                            chain["e"] = e_t
                        elif not chain["init"]:
                            nc.vector.tensor_tensor(s_sl, chain["e"][:], e_t[:], ADD)
                            chain["init"] = True
                            chain["e"] = None
                        else:
                            nc.vector.tensor_tensor(s_sl, s_sl, e_t[:], ADD)

                    for half in range(2):
                        mcs = range(4 * half, 4 * half + 4)
                        accs = {mc: ps.tile([128, rt], F32, tag="acc", name="acc")
                                for mc in mcs}
                        for kc in range(MC):
                            for mc in mcs:
                                nc.tensor.matmul(
                                    accs[mc][:], w2k[kc][:, mc * 128:(mc + 1) * 128],
                                    h1[:, kc, :], start=(kc == 0), stop=(kc == MC - 1))
                        to_add = []
                        for mc in mcs:
                            tpos = min(max(n_pos[lvl] - mc * 128, 0), 128)
                            e_t = epool.tile([128, rt], BF16, tag="e", name="e")
                            if tpos == 128:    # pure w3>=0: relu(z*w3) on ACT
                                nc.scalar.activation(
                                    e_t[:], accs[mc][:], RELU, bias=b2t[:, mc:mc + 1])
                                to_add.append(e_t)
                            elif tpos == 0:    # pure w3<0: min(z*w3, 0) on DVE
                                nc.vector.tensor_scalar(
                                    e_t[:], accs[mc][:], b2t[:, mc:mc + 1], 0.0, ADD, MIN)
                                to_add.append(e_t)
                            elif zero_b2:      # mixed: per-partition clamp
                                nc.vector.tensor_scalar(
                                    e_t[:], accs[mc][:], clot[:, 0:1], chit[:, 0:1],
                                    MAX, MIN)
                                to_add.append(e_t)
                            else:              # mixed, general b2: blend
                                e_p = epool.tile([128, rt], BF16, tag="e", name="e")
                                nc.scalar.activation(
                                    e_p[:], accs[mc][:], RELU, bias=b2t[:, mc:mc + 1])
                                nc.vector.tensor_scalar(
                                    e_t[:], accs[mc][:], b2t[:, mc:mc + 1], 0.0, ADD, MIN)
                                d_t = epool.tile([128, rt], BF16, tag="e", name="e")
                                nc.vector.tensor_tensor(d_t[:], e_p[:], e_t[:],
                                                        mybir.AluOpType.subtract)
                                nc.vector.tensor_scalar(d_t[:], d_t[:], mskt[:, 0:1],
                                                        None, mybir.AluOpType.mult)
                                to_add.append(e_t)
                                to_add.append(d_t)
                        for e_t in to_add:
                            acc_s(e_t)

                    off += rt
                    toff_l += rt
                    n_tile += 1
                pending["v"] = (s_lv, lvl, tiles_l, lvl % 2)
            flush_pending()
    nc.compile()
    return nc


def kernel(x, levels, W1, b1, W2, b2, W3, b3):
    global LAST_RESULTS
    x = np.ascontiguousarray(np.asarray(x, dtype=np.float32))
    levels = np.asarray(levels)
    n = x.shape[0]

    # --- host-side routing: sort rows by level, deal evenly to cores ---
    order = np.argsort(levels, kind="stable")
    counts = np.bincount(np.asarray(levels, dtype=np.int64), minlength=L)[:L]

    # per-level capacity shared by all cores: exactly ceil(count/8); rows are
    # the matmul free dim so no 128-alignment is needed
    caps = [int(-(-int(counts[lvl]) // NC)) for lvl in range(L)]
    r_core = sum(caps)

    # fold W3 into W2/b2, sign-sorted (positives first) per level
    BIG = np.float32(3.0e38)
    W3f = np.asarray(W3, dtype=np.float32)      # [L, H, 1]
    W2f = np.asarray(W2, dtype=np.float32)      # [L, H, H]
    b2f = np.asarray(b2, dtype=np.float32)      # [L, H]
    W2p = np.empty_like(W2f)
    b2p = np.empty_like(b2f)
    clo = np.empty_like(b2f)
    chi = np.empty_like(b2f)
    msk = np.empty_like(b2f)
    n_pos = []
    for lvl in range(L):
        w3 = W3f[lvl, :, 0]
        pos = w3 >= 0
        perm = np.concatenate([np.where(pos)[0], np.where(~pos)[0]])
        n_pos.append(int(pos.sum()))
        W2p[lvl] = (W2f[lvl] * w3[None, :])[:, perm]
        b2p[lvl] = (b2f[lvl] * w3)[perm]
        posp = pos[perm]
        clo[lvl] = np.where(posp, np.float32(0), -BIG)
        chi[lvl] = np.where(posp, BIG, np.float32(0))
        msk[lvl] = posp.astype(np.float32)
    zero_b2 = not np.any(b2p)

    # per-core padded index lists + validity masks
    idx = np.zeros((NC, r_core), dtype=np.int64)
    valid = np.zeros((NC, r_core), dtype=bool)
    lvl_start = np.concatenate([[0], np.cumsum(counts)])
    seg_off = 0
    for lvl in range(L):
        rows = order[lvl_start[lvl]:lvl_start[lvl + 1]]
        nl = len(rows)
        q, rem = divmod(nl, NC)
        start = 0
        for c in range(NC):
            cnt = q + (1 if c < rem else 0)
            idx[c, seg_off:seg_off + cnt] = rows[start:start + cnt]
            valid[c, seg_off:seg_off + cnt] = True
            start += cnt
        seg_off += caps[lvl]

    key = (tuple(caps), tuple(n_pos), zero_b2)
    nc = _PROGRAM_CACHE.get(key)
    if nc is None:
        nc = _build_program(caps, n_pos, zero_b2)
        _PROGRAM_CACHE[key] = nc

    W1b = np.asarray(W1, dtype=np.float32).astype(ml_dtypes.bfloat16)
    W2b = W2p.astype(ml_dtypes.bfloat16)
    b1f = np.asarray(b1, dtype=np.float32)
    in_maps = []
    for c in range(NC):
        xTc = np.ascontiguousarray(x[idx[c]].T).astype(ml_dtypes.bfloat16)  # [D, r_core]
        in_maps.append({
            "xT": xTc,
            "W1": W1b,
            "W2": W2b,
            "b1": b1f,
            "b2": b2p,
            "clo": clo,
            "chi": chi,
            "msk": msk,
        })

    trace = bool(os.environ.get("BASS_KERNEL_TRACE"))
    try:
        res = run_bass_kernel_spmd(nc, in_maps, core_ids=list(range(NC)), trace=trace)
    except Exception:
        # transient NRT_EXEC_UNIT_UNRECOVERABLE wedges have been observed to
        # clear on the next attempt
        import time
        time.sleep(5)
        res = run_bass_kernel_spmd(nc, in_maps, core_ids=list(range(NC)), trace=trace)
    LAST_RESULTS = res

    result = np.zeros((n, 1), dtype=np.float32)
    for c in range(NC):
        oarr = np.asarray(res.results[c]["out"], dtype=np.float32)  # [L, NT_MAX, 512]
        o = np.zeros(r_core, dtype=np.float32)
        pos = 0
        for lvl in range(L):
            toff = 0
            for j, rt in enumerate(_row_tiles(caps[lvl])):
                o[pos + toff:pos + toff + rt] = oarr[lvl, j, 0:rt]
                toff += rt
            pos += caps[lvl]
        result[idx[c][valid[c]], 0] = o[valid[c]]
    result += np.asarray(b3, dtype=np.float32)[np.asarray(levels, dtype=np.int64), :]
    return result


# revision 42
# speedup vs baseline: 1.0073x; 1.0054x over previous
"""Trainium2 Bass kernel for nn_DAInsHead (moe_routing).

Per-row hard-routed 3-layer MLP: rows with levels[i]==l get
    out[i] = W3[l].T @ relu(W2[l].T @ relu(W1[l].T @ x[i] + b1[l]) + b2[l]) + b3[l]

Strategy (vs the reference's dense 4x-redundant masked compute):
  * Host: stable-sort rows by level, deal each level's rows evenly to the 8
    cores, pad each (core, level) segment to cap = ceil(count/8) (exact, no
    128-rounding -- rows are the matmul FREE dim, so no alignment needed),
    and transpose to feature-major xT [D, R_core]. All matmul operands are
    cast to bf16 on the host (total rel err ~4e-3, well under the 2e-2 gate).
  * W3 is folded into W2 on the host: relu(z)*w3 = max(z*w3, 0) for w3>0
    and min(z*w3, 0) for w3<0, so with W2' = W2*w3 (columns sorted by
    sign(w3), positives first) the L2 eviction directly produces each
    column's final contribution; b2' = b2*w3 rides along. This removes the
    entire L3 matvec (8x 512-cycle M=1 matmuls per tile, ~55us/core of PE
    time) from the TensorE.
  * Device (identical SPMD program on 8 cores): per level, keep W1/W2'
    resident in SBUF and stream ~457-511-row tiles: L1/L2 are K=8-chunk
    accumulated 128x128xN bf16 matmuls (1 cycle/row + ~6 cyc/instruction).
    L1 evicts relu(acc+b1) split DVE/ACT. L2 evicts the final signed
    contribution per chunk: Relu-bias on ACT for sign-pure-positive chunks,
    tensor_scalar(ADD,MIN) on DVE for pure-negative, and a per-partition
    (MAX lo, MIN hi) clamp for the one sign-mixed chunk (engine APs must
    start 32-aligned, so the chunk can't be partition-split; clamp needs
    b2==0 -- a masked-blend fallback handles general b2). DVE folds the 8 e
    chunks into a level-wide s[128, cap] accumulator (7 adds per tile,
    emitted after each half's evictions so PSUM release isn't queued behind
    adds on the in-order DVE).
  * Partition reduce of s: per LEVEL, nt back-to-back M=1 ones-matmuls
    (N cycles each at 4x column tiling), emitted between the NEXT level's
    first L1 and L2 matmul groups so the in-order PE never waits on the DVE
    s chains (emitting per tile at tile end measured a 3.6us PE stall per
    tile; per-tile emission pipelined one tile later still cost ~0.6us/tile
    in M=1 enter/exit issue hiccups). Each reduce gets its own PSUM bank at
    partition 0 (start=True clears has_written only for the written region,
    so packing strips 0/32/64 in one bank corrupts the start=False strips).
  * Host: scatter per-core outputs back to original row order, add b3.

Measured on 8xTRN2 (this problem's shapes): 1038us (f32r baseline) -> 968us
(routed bf16, L3 on PE) -> 924.5us (this version; PE busy 96.7%, stream at
the N/2.4GHz+2.5ns bf16 issue-pace roofline, ~874us of pure L1+L2 stream).
Known non-wins (measured): fp8 DoubleRow needs 3-term error compensation to
pass 2e-2 which costs more matmuls than it saves; host-side partition
reduce via per-tile DMA of s[128, rt] slows EVERY matmul ~45ns (SBUF-read
DMA steals moving-operand bandwidth); 256-row first/last tiles + kc-outer
8-bank L1 for tile 0 to soften the cold-DMA startup measured +8us net
(per-instruction overhead of small-N matmuls outweighs the startup gain).
"""
import os
import sys

sys.path.insert(0, "/opt/trn_rl_repo")

import ml_dtypes
import numpy as np

import concourse.bacc as bacc
import concourse.mybir as mybir
import concourse.tile as tile
from concourse.bass_utils import run_bass_kernel_spmd

F32 = mybir.dt.float32
BF16 = mybir.dt.bfloat16
ADD = mybir.AluOpType.add
MAX = mybir.AluOpType.max
MIN = mybir.AluOpType.min
RELU = mybir.ActivationFunctionType.Relu

NC = 8          # cores
L = 4           # levels
D = 1024        # in features
H = 1024        # hidden
KC = D // 128   # contraction chunks
MC = H // 128   # output-feature chunks

LAST_RESULTS = None       # BassKernelResults of the most recent run (for test.py)
_PROGRAM_CACHE = {}


def _row_tiles(cap):
    """Split a per-level capacity into near-equal row tiles of <=512 (PSUM
    bank limit for f32 accumulation)."""
    if cap <= 0:
        return []
    nt = -(-cap // 512)
    base, rem = divmod(cap, nt)
    return [base + 1] * rem + [base] * (nt - rem)


def _build_program(caps, n_pos, zero_b2):
    """Build + compile the SPMD program for per-level capacities `caps` and
    per-level positive-w3 column counts `n_pos` (W2 columns are host-sorted
    so cols [0, n_pos) have w3>=0 and [n_pos, H) have w3<0). `zero_b2`
    selects a 1-op per-partition clamp for the one sign-mixed chunk per
    level (engine APs must start 32-aligned, so the chunk can't be split at
    an arbitrary partition)."""
    r_core = sum(caps)
    nc = bacc.Bacc("TRN2", target_bir_lowering=False, debug=False, num_devices=NC)
    xT = nc.dram_tensor("xT", [D, r_core], BF16, kind="ExternalInput")
    W1 = nc.dram_tensor("W1", [L, D, H], BF16, kind="ExternalInput")
    W2 = nc.dram_tensor("W2", [L, H, H], BF16, kind="ExternalInput")  # pre-folded W2*w3, sign-sorted
    b1 = nc.dram_tensor("b1", [L, H], F32, kind="ExternalInput")
    b2 = nc.dram_tensor("b2", [L, H], F32, kind="ExternalInput")      # pre-folded b2*w3, sign-sorted
    clo = nc.dram_tensor("clo", [L, H], F32, kind="ExternalInput")    # 0 where w3>=0 else -BIG
    chi = nc.dram_tensor("chi", [L, H], F32, kind="ExternalInput")    # +BIG where w3>=0 else 0
    msk = nc.dram_tensor("msk", [L, H], F32, kind="ExternalInput")    # 1 where w3>=0 else 0
    # Per-level partition-reduce results: row j = tile j's [1, rt] sums,
    # padded to 512 (junk beyond each tile's width; host slices valid parts).
    # Per-tile DMA of the full s[128, rt] for a host-side reduce was tried
    # and measured +45ns on EVERY matmul (SBUF-read DMA steals moving-operand
    # bandwidth) -- keep output DMA tiny.
    NT_MAX = max(len(_row_tiles(c)) for c in caps if c) if any(caps) else 1
    out = nc.dram_tensor("out", [L, NT_MAX, 512], F32, kind="ExternalOutput")

    xT_r = xT.rearrange("(kc p) r -> p kc r", p=128)  # [128, KC, r_core]

    with tile.TileContext(nc) as tc:
        with (
            tc.tile_pool(name="wpool", bufs=2) as wpool,
            tc.tile_pool(name="bpool", bufs=2) as bpool,
            tc.tile_pool(name="xpool", bufs=2) as xpool,
            tc.tile_pool(name="hpool", bufs=2) as hpool,
            tc.tile_pool(name="epool", bufs=10) as epool,
            tc.tile_pool(name="spool", bufs=2) as spool,
            tc.tile_pool(name="opool", bufs=4) as opool,
            tc.tile_pool(name="ps", bufs=8, space="PSUM") as ps,
        ):
            ones = bpool.tile([128, NT_MAX], BF16, tag="ones", bufs=1)
            nc.gpsimd.memset(ones[:], 1.0)

            # Deferred per-LEVEL partition reduce: level l's ones-matmuls are
            # emitted between level l+1's first-tile L1 and L2 groups, so the
            # in-order PE never waits on the DVE s chains. The nt matmuls for
            # a level run back-to-back into DISJOINT partition rows of one
            # PSUM bank (start only on the first: start=True clears the whole
            # bank), so the ~0.3us enter/exit issue hiccup around M=1
            # matmuls is paid once per level, not once per tile.
            pending = {}

            def flush_pending():
                if not pending:
                    return
                s_lv, p_lvl, tlist, parity = pending.pop("v")
                # One bank per tile-reduce, output at partition 0, plain
                # start&stop semantics. (Packing 3 strips at 0/32/64 into a
                # shared bank with start only on the first was tried:
                # start=True clears has_written only for the written region,
                # so the start=False strips accumulated onto stale PSUM.)
                toff = 0
                for j, rt_j in enumerate(tlist):
                    b_ps = ps.tile([128, 512], F32, tag="acc", name="acc")
                    nc.tensor.matmul(b_ps[0:1, 0:rt_j], ones[:, j:j + 1],
                                     s_lv[:, toff:toff + rt_j],
                                     start=True, stop=True)
                    o_t = opool.tile([1, 512], F32, tag="o", name="o")
                    if (j + parity) % 2 == 0:
                        nc.vector.tensor_scalar(
                            o_t[0:1, :], b_ps[0:1, 0:512], 0.0, None, ADD)
                    else:
                        nc.scalar.copy(o_t[0:1, :], b_ps[0:1, 0:512])
                    q = nc.gpsimd if (j + parity) % 2 == 0 else nc.sync
                    q.dma_start(out[p_lvl][j:j + 1, :], o_t[0:1, :])
                    toff += rt_j

            off = 0
            n_tile = 0
            for lvl in range(L):
                cap = caps[lvl]
                if cap == 0:
                    continue
                tiles_l = _row_tiles(cap)
                # For level 0, issue the first row-tile's x DMA before the
                # weight DMAs so the PE can start as soon as the first weight
                # chunk lands instead of waiting behind 8.5MB of weights.
                pre_x = None
                if lvl == 0:
                    rt0 = tiles_l[0]
                    pre_x = xpool.tile([128, KC, rt0], BF16, tag="x")
                    # first x chunk only -- the very first matmul needs just
                    # this chunk plus w1k[0], so those two DMAs go first
                    nc.sync.dma_start(pre_x[:, 0, :], xT_r[:, 0, 0:rt0])
                # Startup DMA issue stays on the single sync queue: spreading
                # level-0 issues across scalar/gpsimd rings was tried and
                # measured +6us -- W2's transfers then compete with W1's for
                # the ramping HBM bandwidth and delay the L1-critical chunks.
                q_x = nc.sync
                q_b = nc.sync
                w1k = []
                w2k = []
                t1 = wpool.tile([128, H], BF16, tag="w1k0")
                nc.sync.dma_start(t1[:], W1[lvl][0:128, :])
                w1k.append(t1)
                if lvl == 0:
                    for kc in range(1, KC):
                        q_x.dma_start(pre_x[:, kc, :], xT_r[:, kc, 0:rt0])
                # Tiny bias tiles before the bulk of W1/W2 so evictions
                # never wait behind 8MB of weight DMA.
                b1t = bpool.tile([128, MC], F32, tag="b1")
                q_b.dma_start(b1t[:], b1[lvl].rearrange("(mc p) -> p mc", p=128))
                b2t = bpool.tile([128, MC], F32, tag="b2")
                q_b.dma_start(b2t[:], b2[lvl].rearrange("(mc p) -> p mc", p=128))
                mix_mc = n_pos[lvl] // 128 if n_pos[lvl] % 128 else -1
                if mix_mc >= 0:
                    if zero_b2:
                        clot = bpool.tile([128, 1], F32, tag="clo")
                        q_b.dma_start(
                            clot[:], clo[lvl].rearrange("(mc p) -> p mc", p=128)[:, mix_mc:mix_mc + 1])
                        chit = bpool.tile([128, 1], F32, tag="chi")
                        q_b.dma_start(
                            chit[:], chi[lvl].rearrange("(mc p) -> p mc", p=128)[:, mix_mc:mix_mc + 1])
                    else:
                        mskt = bpool.tile([128, 1], F32, tag="msk")
                        q_b.dma_start(
                            mskt[:], msk[lvl].rearrange("(mc p) -> p mc", p=128)[:, mix_mc:mix_mc + 1])
                # Per-kc weight tiles so the first matmuls only wait on the
                # first 256KB of weight DMA, and level l+1 prefetch
                # double-buffers against level l (bufs=2 per tag).
                for kc in range(1, KC):
                    t1 = wpool.tile([128, H], BF16, tag=f"w1k{kc}")
                    nc.sync.dma_start(t1[:], W1[lvl][kc * 128:(kc + 1) * 128, :])
                    w1k.append(t1)
                for kc in range(KC):
                    t2 = wpool.tile([128, H], BF16, tag=f"w2k{kc}")
                    nc.sync.dma_start(t2[:], W2[lvl][kc * 128:(kc + 1) * 128, :])
                    w2k.append(t2)

                # one level-wide s accumulator; each tile's chain fills its
                # column slice, the level flush reduces it over partitions
                s_lv = spool.tile([128, cap], BF16, tag="s", name="s")
                toff_l = 0
                for ti, rt in enumerate(tiles_l):
                    if pre_x is not None and ti == 0:
                        x_t = pre_x
                    else:
                        x_t = xpool.tile([128, KC, rt], BF16, tag="x")
                        # per-kc chunk DMAs: the tile's first matmul needs
                        # only chunk 0, and the chunks interleave with the
                        # next level's 4MB weight prefetch on the shared
                        # queue instead of queueing behind it
                        for kc in range(KC):
                            nc.sync.dma_start(x_t[:, kc, :],
                                              xT_r[:, kc, off:off + rt])

                    # L1 runs kc-outer in two 4-bank halves: the first matmul
                    # only depends on w1k[0] + x_t, so the PE ramps with the
                    # weight DMA stream instead of waiting for all of W1.
                    h1 = hpool.tile([128, MC, rt], BF16, tag="h1")
                    for half in range(2):
                        mcs = range(4 * half, 4 * half + 4)
                        accs = {mc: ps.tile([128, rt], F32, tag="acc", name="acc")
                                for mc in mcs}
                        for kc in range(KC):
                            for mc in mcs:
                                nc.tensor.matmul(
                                    accs[mc][:], w1k[kc][:, mc * 128:(mc + 1) * 128],
                                    x_t[:, kc, :], start=(kc == 0), stop=(kc == KC - 1))
                        # evictions split DVE/ACT so PSUM banks release fast
                        # (bank reuse gates the next matmul group's start);
                        # DVE takes 1 of 4 since it also carries the L2
                        # e-chunk accumulation below
                        for mc in mcs:
                            if mc % 4 == 0:
                                nc.vector.tensor_scalar(
                                    h1[:, mc, :], accs[mc][:], b1t[:, mc:mc + 1], 0.0, ADD, MAX)
                            else:
                                nc.scalar.activation(
                                    h1[:, mc, :], accs[mc][:], RELU, bias=b1t[:, mc:mc + 1])

                    # previous level's partition reduce goes here, between
                    # this level's first L1 and L2 matmul groups on the PE
                    if ti == 0:
                        flush_pending()

                    # L2 with W3 folded in: eviction yields each column's
                    # final signed contribution; DVE folds chunks into s.
                    # Engine APs must start at 32-aligned partitions, so
                    # chunks are never partition-split: sign-pure chunks use
                    # Relu-with-bias (ACT) / (ADD,MIN) (DVE); the one mixed
                    # chunk uses a per-partition (MAX lo, MIN hi) clamp
                    # (b2==0) or a full-width masked blend (general b2).
                    # Per half: all PSUM evictions are emitted BEFORE the s
                    # adds, so bank release (which gates the next matmul
                    # group) isn't queued behind adds on the in-order DVE.
                    s_sl = s_lv[:, toff_l:toff_l + rt]
                    chain = {"init": False, "e": None}

                    def acc_s(e_t, chain=chain, s_sl=s_sl):
                        if not chain["init"] and chain["e"] is None:
                            chain["e"] = e_t
                        elif not chain["init"]:
                            nc.vector.tensor_tensor(s_sl, chain["e"][:], e_t[:], ADD)
                            chain["init"] = True
                            chain["e"] = None
                        else:
                            nc.vector.tensor_tensor(s_sl, s_sl, e_t[:], ADD)

                    for half in range(2):
                        mcs = range(4 * half, 4 * half + 4)
                        accs = {mc: ps.tile([128, rt], F32, tag="acc", name="acc")
                                for mc in mcs}
                        for kc in range(MC):
                            for mc in mcs:
                                nc.tensor.matmul(
                                    accs[mc][:], w2k[kc][:, mc * 128:(mc + 1) * 128],
                                    h1[:, kc, :], start=(kc == 0), stop=(kc == MC - 1))
                        to_add = []
                        for mc in mcs:
                            tpos = min(max(n_pos[lvl] - mc * 128, 0), 128)
                            e_t = epool.tile([128, rt], BF16, tag="e", name="e")
                            if tpos == 128:    # pure w3>=0: relu(z*w3) on ACT
                                nc.scalar.activation(
                                    e_t[:], accs[mc][:], RELU, bias=b2t[:, mc:mc + 1])
                                to_add.append(e_t)
                            elif tpos == 0:    # pure w3<0: min(z*w3, 0) on DVE
                                nc.vector.tensor_scalar(
                                    e_t[:], accs[mc][:], b2t[:, mc:mc + 1], 0.0, ADD, MIN)
                                to_add.append(e_t)
                            elif zero_b2:      # mixed: per-partition clamp
                                nc.vector.tensor_scalar(
                                    e_t[:], accs[mc][:], clot[:, 0:1], chit[:, 0:1],
                                    MAX, MIN)
                                to_add.append(e_t)
                            else:              # mixed, general b2: blend
                                e_p = epool.tile([128, rt], BF16, tag="e", name="e")
                                nc.scalar.activation(
                                    e_p[:], accs[mc][:], RELU, bias=b2t[:, mc:mc + 1])
                                nc.vector.tensor_scalar(
                                    e_t[:], accs[mc][:], b2t[:, mc:mc + 1], 0.0, ADD, MIN)
                                d_t = epool.tile([128, rt], BF16, tag="e", name="e")
                                nc.vector.tensor_tensor(d_t[:], e_p[:], e_t[:],
                                                        mybir.AluOpType.subtract)
                                nc.vector.tensor_scalar(d_t[:], d_t[:], mskt[:, 0:1],
                                                        None, mybir.AluOpType.mult)
                                to_add.append(e_t)
                                to_add.append(d_t)
                        for e_t in to_add:
                            acc_s(e_t)

                    off += rt
                    toff_l += rt
                    n_tile += 1
                pending["v"] = (s_lv, lvl, tiles_l, lvl % 2)
            flush_pending()
    nc.compile()
    return nc


def kernel(x, levels, W1, b1, W2, b2, W3, b3):
    global LAST_RESULTS
    x = np.ascontiguousarray(np.asarray(x, dtype=np.float32))
    levels = np.asarray(levels)
    n = x.shape[0]

    # --- host-side routing: sort rows by level, deal evenly to cores ---
    order = np.argsort(levels, kind="stable")
    counts = np.bincount(np.asarray(levels, dtype=np.int64), minlength=L)[:L]

    # per-level capacity shared by all cores: exactly ceil(count/8); rows are
    # the matmul free dim so no 128-alignment is needed
    caps = [int(-(-int(counts[lvl]) // NC)) for lvl in range(L)]
    r_core = sum(caps)

    # fold W3 into W2/b2, sign-sorted (positives first) per level
    BIG = np.float32(3.0e38)
    W3f = np.asarray(W3, dtype=np.float32)      # [L, H, 1]
    W2f = np.asarray(W2, dtype=np.float32)      # [L, H, H]
    b2f = np.asarray(b2, dtype=np.float32)      # [L, H]
    W2p = np.empty_like(W2f)
    b2p = np.empty_like(b2f)
    clo = np.empty_like(b2f)
    chi = np.empty_like(b2f)
    msk = np.empty_like(b2f)
    n_pos = []
    for lvl in range(L):
        w3 = W3f[lvl, :, 0]
        pos = w3 >= 0
        perm = np.concatenate([np.where(pos)[0], np.where(~pos)[0]])
        n_pos.append(int(pos.sum()))
        W2p[lvl] = (W2f[lvl] * w3[None, :])[:, perm]
        b2p[lvl] = (b2f[lvl] * w3)[perm]
        posp = pos[perm]
        clo[lvl] = np.where(posp, np.float32(0), -BIG)
        chi[lvl] = np.where(posp, BIG, np.float32(0))
        msk[lvl] = posp.astype(np.float32)
    zero_b2 = not np.any(b2p)

    # per-core padded index lists + validity masks
    idx = np.zeros((NC, r_core), dtype=np.int64)
    valid = np.zeros((NC, r_core), dtype=bool)
    lvl_start = np.concatenate([[0], np.cumsum(counts)])
    seg_off = 0
    for lvl in range(L):
        rows = order[lvl_start[lvl]:lvl_start[lvl + 1]]
        nl = len(rows)
        q, rem = divmod(nl, NC)
        start = 0
        for c in range(NC):
            cnt = q + (1 if c < rem else 0)
            idx[c, seg_off:seg_off + cnt] = rows[start:start + cnt]
            valid[c, seg_off:seg_off + cnt] = True
            start += cnt
        seg_off += caps[lvl]

    key = (tuple(caps), tuple(n_pos), zero_b2)
    nc = _PROGRAM_CACHE.get(key)
    if nc is None:
        nc = _build_program(caps, n_pos, zero_b2)
        _PROGRAM_CACHE[key] = nc

    W1b = np.asarray(W1, dtype=np.float32).astype(ml_dtypes.bfloat16)
    W2b = W2p.astype(ml_dtypes.bfloat16)
    b1f = np.asarray(b1, dtype=np.float32)
    in_maps = []
    for c in range(NC):
        xTc = np.ascontiguousarray(x[idx[c]].T).astype(ml_dtypes.bfloat16)  # [D, r_core]
        in_maps.append({
            "xT": xTc,
            "W1": W1b,
            "W2": W2b,
            "b1": b1f,
            "b2": b2p,
            "clo": clo,
            "chi": chi,
            "msk": msk,
        })

    trace = bool(os.environ.get("BASS_KERNEL_TRACE"))
    try:
        res = run_bass_kernel_spmd(nc, in_maps, core_ids=list(range(NC)), trace=trace)
    except Exception:
        # transient NRT_EXEC_UNIT_UNRECOVERABLE wedges have been observed to
        # clear on the next attempt
        import time
        time.sleep(5)
        res = run_bass_kernel_spmd(nc, in_maps, core_ids=list(range(NC)), trace=trace)
    LAST_RESULTS = res

    result = np.zeros((n, 1), dtype=np.float32)
    for c in range(NC):
        oarr = np.asarray(res.results[c]["out"], dtype=np.float32)  # [L, NT_MAX, 512]
        o = np.zeros(r_core, dtype=np.float32)
        pos = 0
        for lvl in range(L):
            toff = 0
            for j, rt in enumerate(_row_tiles(caps[lvl])):
                o[pos + toff:pos + toff + rt] = oarr[lvl, j, 0:rt]
                toff += rt
            pos += caps[lvl]
        result[idx[c][valid[c]], 0] = o[valid[c]]
    result += np.asarray(b3, dtype=np.float32)[np.asarray(levels, dtype=np.int64), :]
    return result
